# revision 9
# baseline (speedup 1.0000x reference)
"""SAGAN-style attention block on 8 TRN2 NeuronCores, data-parallel over batch.

Per core (one batch b): x_b [C=256, N=4096] f32.
  q = Wq x + bq  [32, N];  k = Wk x + bk  [32, N]
  S = q^T k  [N, N];  attn = softmax(S, axis=0)  (column softmax over i)
  out = gamma * (v @ attn) + x,  v = Wv x + bv

Device algorithm (bf16 matmuls, f32 PSUM accumulation):
  - no max-subtraction in softmax: |S| < ~50 empirically, exp() fits f32/bf16
  - bv folded into the residual: out = gamma*(v0@E)/s + (x + gamma*bv),
    v0 = Wv x (no bias), E = exp(S), s = column sums of E
  - colsum s computed via an all-ones(1/gamma) [128,128] stationary matmul
    chain -> PSUM holds s/gamma broadcast in every partition
  - q/k replicated 4x across partitions so K=32 qk matmuls can be
    row-tiled (tile_position) into distinct 32-row groups of the PE array
"""

import numpy as np
import ml_dtypes

import concourse.bass as bass
import concourse.mybir as mybir
from concourse import bacc, tile
from concourse.bass import ds
from concourse.bass_utils import run_bass_kernel_spmd

F32 = mybir.dt.float32
BF16 = mybir.dt.bfloat16
AF = mybir.ActivationFunctionType

B, C, N = 8, 256, 4096
C8 = 32
P = 128
JT = 512          # j-tile width
NJT = N // JT     # 8 j-tiles
NKC = N // P      # 32 i/k chunks of 128

_cache = {}


def _build_nc():
    nc = bacc.Bacc("TRN2", target_bir_lowering=False, debug=False, num_devices=B)

    x_d = nc.dram_tensor("x", [C, N], F32, kind="ExternalInput").ap()
    x16_d = nc.dram_tensor("x16", [C, N], BF16, kind="ExternalInput").ap()
    wq_d = nc.dram_tensor("wq", [P, 2, P], BF16, kind="ExternalInput").ap()
    wk_d = nc.dram_tensor("wk", [P, 2, P], BF16, kind="ExternalInput").ap()
    wvt_d = nc.dram_tensor("wvt", [P, 2, C], BF16, kind="ExternalInput").ap()
    bq_d = nc.dram_tensor("bq", [P, 1], F32, kind="ExternalInput").ap()
    bk_d = nc.dram_tensor("bk", [P, 1], F32, kind="ExternalInput").ap()
    gbv_d = nc.dram_tensor("gbv", [P, 2], F32, kind="ExternalInput").ap()
    invg_d = nc.dram_tensor("invg", [P, P], BF16, kind="ExternalInput").ap()
    out_d = nc.dram_tensor("out", [C, N], F32, kind="ExternalOutput").ap()

    x_view = x_d.rearrange("(o p) n -> p o n", p=P)
    out_view = out_d.rearrange("(o p) n -> p o n", p=P)
    x16_view = x16_d.rearrange("(o p) n -> p o n", p=P)

    with tile.TileContext(nc) as tc:
        with (
            tc.tile_pool(name="const", bufs=1) as cpool,
            tc.tile_pool(name="big", bufs=1) as bigpool,
            tc.tile_pool(name="epool", bufs=2) as epool,
            tc.tile_pool(name="work", bufs=2) as wpool,
            tc.tile_pool(name="psA", bufs=2, space="PSUM") as psA,
            tc.tile_pool(name="psU", bufs=2, space="PSUM") as psU,
            tc.tile_pool(name="psS", bufs=1, space="PSUM") as psS,
        ):
            # ---- constants ----
            wq_sb = cpool.tile([P, 2, P], BF16, tag="wq")
            nc.sync.dma_start(out=wq_sb[:], in_=wq_d)
            wk_sb = cpool.tile([P, 2, P], BF16, tag="wk")
            nc.sync.dma_start(out=wk_sb[:], in_=wk_d)
            wvt_sb = cpool.tile([P, 2, C], BF16, tag="wvt")
            nc.sync.dma_start(out=wvt_sb[:], in_=wvt_d)
            bq_sb = cpool.tile([P, 1], F32, tag="bq")
            nc.sync.dma_start(out=bq_sb[:], in_=bq_d)
            bk_sb = cpool.tile([P, 1], F32, tag="bk")
            nc.sync.dma_start(out=bk_sb[:], in_=bk_d)
            gbv_sb = cpool.tile([P, 2], F32, tag="gbv")
            nc.sync.dma_start(out=gbv_sb[:], in_=gbv_d)
            invg_sb = cpool.tile([P, P], BF16, tag="invg")
            nc.sync.dma_start(out=invg_sb[:], in_=invg_d)

            # ---- big per-batch tensors ----
            x16_sb = bigpool.tile([P, 2, N], BF16, tag="x16")
            for dch in range(4):
                nc.sync.dma_start(
                    out=x16_sb[:, :, ds(dch * (N // 4), N // 4)],
                    in_=x16_view[:, :, ds(dch * (N // 4), N // 4)],
                )
            x_sb = bigpool.tile([P, 2, N], F32, tag="x")
            nc.sync.dma_start(out=x_sb[:], in_=x_view)
            # xb = x + gamma*bv  (residual with folded v-bias), in place
            nc.vector.tensor_add(
                out=x_sb[:],
                in0=x_sb[:],
                in1=gbv_sb[:, :, None].to_broadcast((P, 2, N)),
            )

            q_sb = bigpool.tile([P, N], BF16, tag="q")
            k_sb = bigpool.tile([P, N], BF16, tag="k")
            vt_sb = bigpool.tile([P, NKC, C], BF16, tag="vt")

            # ---- projections ----
            # q/k: out[m, n] = sum_c W_rep[c, m] * x16[c, n]  (M=128: 4 replicas)
            for (w_sb, b_sb, dst) in ((wq_sb, bq_sb, q_sb), (wk_sb, bk_sb, k_sb)):
                for g in range(4):
                    ps = psA.tile([P, 2, JT], F32, tag="psA")
                    for t2 in range(2):
                        nch = 2 * g + t2
                        for kc in range(2):
                            nc.tensor.matmul(
                                ps[:, t2, :],
                                w_sb[:, kc, :],
                                x16_sb[:, kc, ds(nch * JT, JT)],
                                start=(kc == 0),
                                stop=(kc == 1),
                            )
                    nc.vector.tensor_add(
                        out=dst[:, ds(g * 2 * JT, 2 * JT)],
                        in0=ps[:].rearrange("p a b -> p (a b)"),
                        in1=b_sb[:, :].to_broadcast((P, 2 * JT)),
                    )
            # vT: out[n, c] = sum_c' x16[c', n] * WvT[c', c]
            for g in range(16):
                ps = psA.tile([P, 2, JT], F32, tag="psA")
                for t2 in range(2):
                    nck = 2 * g + t2
                    for kc in range(2):
                        nc.tensor.matmul(
                            ps[:, t2, :C],
                            x16_sb[:, kc, ds(nck * P, P)],
                            wvt_sb[:, kc, :],
                            start=(kc == 0),
                            stop=(kc == 1),
                        )
                nc.any.tensor_copy(out=vt_sb[:, ds(2 * g, 2), :], in_=ps[:, :, :C])

            # ---- main software-pipelined loop over j-tiles ----
            # step t: emit colsum chain for tile t-1, then interleaved
            # [qk+exp of tile t | v@E of tile t-1], then normalize+store t-1.
            e_tiles = {}
            ep_tiles = {}
            s_ps_t = {}
            u_ps_t = {}

            for step in range(NJT + 1):
                t_prev = step - 1

                # colsum of tile t-1 over pair-summed E: s/gamma broadcast
                if step >= 1:
                    ep_prev = ep_tiles[t_prev]
                    sps = psS.tile([P, JT], F32, tag="s")
                    s_ps_t[t_prev] = sps
                    for kc in range(NKC // 4):
                        nc.tensor.matmul(
                            sps[:],
                            invg_sb[:],
                            ep_prev[:, kc, :],
                            start=(kc == 0),
                            stop=(kc == NKC // 4 - 1),
                        )
                    r_sb = wpool.tile([P, JT], F32, tag="r")
                    rscr = wpool.tile([P, JT], F32, tag="rscr")
                    nc.vector.reciprocal_approx_accurate(
                        out=r_sb[:], in_=sps[:], scratch=rscr[:]
                    )
                    s_ps_t[t_prev] = (sps, r_sb)

                if step < NJT:
                    e_tiles[step] = epool.tile([P, NKC, JT], BF16, tag="E", name=f"E_{step}")
                    ep_tiles[step] = epool.tile(
                        [P, NKC // 2, JT], BF16, tag="Epair", name=f"Ep_{step}"
                    )
                if step >= 1:
                    u_ps_t[t_prev] = [
                        psU.tile([P, JT], F32, tag="u", name=f"u_{t_prev}_{m}")
                        for m in range(2)
                    ]

                for g in range(NKC // 2):
                    if step < NJT:
                        js = ds(step * JT, JT)
                        e_cur = e_tiles[step]
                        ps = psA.tile([P, 2, JT], F32, tag="psA")
                        for j2 in range(2):
                            ic = 2 * g + j2
                            trow = ic % 4
                            nc.tensor.matmul(
                                ps[:, j2, :],
                                q_sb[32 * trow : 32 * (trow + 1), ds(ic * P, P)],
                                k_sb[32 * trow : 32 * (trow + 1), js],
                                start=True,
                                stop=True,
                                tile_position=(32 * trow, 0),
                            )
                        nc.scalar.activation(
                            e_cur[:, ds(2 * g, 2), :], ps[:], AF.Exp
                        )
                        # pair/quad-sum tree for the colsum chain (quarters
                        # its PE matmuls); quads overwrite ep rows in place
                        nc.vector.tensor_add(
                            out=ep_tiles[step][:, g, :],
                            in0=e_cur[:, 2 * g, :],
                            in1=e_cur[:, 2 * g + 1, :],
                        )
                        if g % 2 == 1:
                            h = g // 2
                            nc.vector.tensor_add(
                                out=ep_tiles[step][:, h, :],
                                in0=ep_tiles[step][:, 2 * h, :],
                                in1=ep_tiles[step][:, 2 * h + 1, :],
                            )
                    if step >= 1:
                        e_prev = e_tiles[t_prev]
                        for m in range(2):
                            for j2 in range(2):
                                kc = 2 * g + j2
                                nc.tensor.matmul(
                                    u_ps_t[t_prev][m][:],
                                    vt_sb[:, kc, ds(m * P, P)],
                                    e_prev[:, kc, :],
                                    start=(kc == 0),
                                    stop=(kc == NKC - 1),
                                )

                # normalize + residual + store tile t-1
                if step >= 1:
                    js_prev = ds(t_prev * JT, JT)
                    _, r_sb = s_ps_t[t_prev]
                    outt = wpool.tile([P, 2, JT], F32, tag="outt")
                    for m in range(2):
                        tmp = wpool.tile([P, JT], F32, tag="tmp")
                        nc.vector.tensor_mul(
                            out=tmp[:], in0=u_ps_t[t_prev][m][:], in1=r_sb[:]
                        )
                        nc.vector.tensor_add(
                            out=outt[:, m, :],
                            in0=tmp[:],
                            in1=x_sb[:, m, js_prev],
                        )
                    nc.sync.dma_start(out=out_view[:, :, js_prev], in_=outt[:])

    nc.compile()
    return nc


def _prep_inputs(x, Wq, bq, Wk, bk, Wv, bv, gamma):
    x = np.asarray(x, dtype=np.float32)
    Wq = np.asarray(Wq, dtype=np.float32)
    bq = np.asarray(bq, dtype=np.float32)
    Wk = np.asarray(Wk, dtype=np.float32)
    bk = np.asarray(bk, dtype=np.float32)
    Wv = np.asarray(Wv, dtype=np.float32)
    bv = np.asarray(bv, dtype=np.float32)
    g = float(np.asarray(gamma))

    bf = ml_dtypes.bfloat16
    # WqT replicated 4x along M so q lands replicated across 4x32 partitions
    wq_rep = np.tile(Wq.T, (1, 4)).reshape(2, P, P).transpose(1, 0, 2)
    wk_rep = np.tile(Wk.T, (1, 4)).reshape(2, P, P).transpose(1, 0, 2)
    wvt = Wv.T.reshape(2, P, C).transpose(1, 0, 2)
    bq_rep = np.tile(bq, 4)[:, None].astype(np.float32)
    bk_rep = np.tile(bk, 4)[:, None].astype(np.float32)
    gbv = (g * bv).reshape(2, P).T.copy().astype(np.float32)
    inv_g = 1.0 / g if g != 0.0 else 0.0
    invg = np.full((P, P), inv_g, dtype=np.float32)

    xf = x.reshape(B, C, N)
    x16 = xf.astype(bf)

    shared = {
        "wq": np.ascontiguousarray(wq_rep.astype(bf)),
        "wk": np.ascontiguousarray(wk_rep.astype(bf)),
        "wvt": np.ascontiguousarray(wvt.astype(bf)),
        "bq": bq_rep,
        "bk": bk_rep,
        "gbv": gbv,
        "invg": invg.astype(bf),
    }
    in_maps = []
    for b in range(B):
        m = dict(shared)
        m["x"] = np.ascontiguousarray(xf[b])
        m["x16"] = np.ascontiguousarray(x16[b])
        in_maps.append(m)
    return in_maps


def _get_nc():
    if "nc" not in _cache:
        _cache["nc"] = _build_nc()
    return _cache["nc"]


def _run(in_maps, trace=False):
    nc = _get_nc()
    return run_bass_kernel_spmd(nc, in_maps, core_ids=list(range(B)), trace=trace)


def kernel(x, Wq, bq, Wk, bk, Wv, bv, gamma, _trace=False):
    x = np.asarray(x, dtype=np.float32)
    in_maps = _prep_inputs(x, Wq, bq, Wk, bk, Wv, bv, gamma)
    res = _run(in_maps, trace=_trace)
    out = np.stack([res.results[b]["out"] for b in range(B)])
    out = out.reshape(x.shape).astype(np.float32)
    if _trace:
        return out, res
    return out


def _enable_ntff_hook():
    """Register the axon NTFF profile hook (missing antenv.axon_hooks shim)."""
    import sys, types

    if "antenv.axon_hooks" in sys.modules:
        return
    mod = types.ModuleType("antenv.axon_hooks")
    mod._hook = None
    mod.set_axon_ntff_profile_hook = lambda h: setattr(mod, "_hook", h)
    mod.get_axon_ntff_profile_hook = lambda: mod._hook
    sys.modules["antenv.axon_hooks"] = mod
    import antenv

    antenv.axon_hooks = mod
    from trn_agent_boot.trn_boot import _ntff_profile_via_ctypes

    mod._hook = _ntff_profile_via_ctypes("/opt/axon/libaxon_pjrt.so")
